# revision 32
# baseline (speedup 1.0000x reference)
"""Multi-head attention (shared key head) on 8 TRN2 NeuronCores — v4.

Sharding: core c handles batch b = c % 4 and head group g = c // 4
(heads 4g..4g+3).  Per-core weights are sliced on host; x is
pre-transposed (and bf16-cast) on host.  sqrt(scale) is folded into
Wq/bq/Wk on host so scores arrive pre-scaled.

v4: the exp stream on the scalar engine (ACT, ~1.2G col/s) is the
serial critical path (~70us if unassisted), so the whole kernel is
re-scheduled around it:
  - units ordered largest-exp-first: (0,3),(1,3),(0,2),(1,2),...
    so ACT runs flat-out from ~t=9us; projections and AV chains are
    woven between score-blocks as PE filler at the pace ACT drains
    score PSUM tiles (ps_sc is double-buffered; the in-order PE queue
    self-regulates).
  - AV chain for unit X is emitted two units later (its exps are
    certainly done: no PE stall).
  - a subset of early off-diag blocks run quadratic exp on the DVE
    (exp(s) ~= (1+s) + s^2/2, |s|<=0.29 so err < 4e-3) using only
    2x-capable ops: copy + tensor_scalar + tensor_tensor.
  - softmax normalisation on the host: the ones-column of V gives the
    denominator as row 64 of the AV accumulator; device ships raw
    [65, q] fp32 per (head, chunk); host divides + transposes.
  - x DMA'd in [128,512] chunk-granular pieces, ordered so kt(0) and
    qt(0,3) fire ~6us in; K projection uses a host-duplicated
    [D, 128] Wk (full-width matmul, single PSUM->SBUF copy).
"""

import math
import numpy as np
import ml_dtypes

import concourse.bass as bass
import concourse.mybir as mybir
import concourse.tile as tile
from concourse import bacc
from concourse.bass_utils import run_bass_kernel_spmd
from concourse.alu_op_type import AluOpType

B, S, D = 4, 2048, 512
H, A, O = 8, 64, 64
NCORES = 8
HPC = 4                # heads per core
APC = HPC * A          # 256 projection cols per core
VB = O + 1             # V block: 64 out + 1 ones column (denominator)
SCALE = 1.0 / math.sqrt(S)
RS = math.sqrt(SCALE)  # folded into wq, bq, wk on host

F32 = mybir.dt.float32
BF16 = mybir.dt.bfloat16
AF = mybir.ActivationFunctionType
BF_NP = ml_dtypes.bfloat16

QCH = 512              # q-chunk width
NCH = S // QCH         # 4
N_DT = D // 128        # 4 contraction tiles
N_ST = S // 128        # 16 s-tiles / k-tiles

# off-diagonal score blocks whose exp runs on the DVE (quadratic)
DVE_EXP = {(0, 3): (0, 4, 8), (1, 3): (0, 4, 8)}


def build():
    nc = bacc.Bacc("TRN2", target_bir_lowering=False, debug=False,
                   num_devices=NCORES)

    xT_d = nc.dram_tensor("xT4", [NCH, D, 512], BF16,
                          kind="ExternalInput").ap()
    wq_d = nc.dram_tensor("wq", [D, APC], BF16, kind="ExternalInput").ap()
    bq_d = nc.dram_tensor("bq", [2, 128, 1], F32, kind="ExternalInput").ap()
    wk2_d = nc.dram_tensor("wk2", [D, 128], BF16, kind="ExternalInput").ap()
    wv_d = nc.dram_tensor("wv", [D, APC], BF16, kind="ExternalInput").ap()
    bvm_d = nc.dram_tensor("bvm", [128, HPC * O], BF16,
                           kind="ExternalInput").ap()
    out_d = nc.dram_tensor("out", [HPC, NCH, VB, QCH], F32,
                           kind="ExternalOutput").ap()

    # causal boundary-block mask: tri[k, q] = 1 where q >= k (valid),
    # duplicated for both heads of a pair ([128, 2*128] -> [p, 2, 128])
    tri_np = (np.arange(128)[None, :] >= np.arange(128)[:, None])
    tri2_np = np.concatenate([tri_np, tri_np], axis=1)
    tri_d = nc.inline_tensor(tri2_np.astype(BF_NP), "tri").ap()

    with tile.TileContext(nc) as tc:
        with tc.tile_pool(name="const", bufs=1) as cpool, \
             tc.tile_pool(name="persist", bufs=1) as ppool, \
             tc.tile_pool(name="attn", bufs=48) as apool, \
             tc.tile_pool(name="fin", bufs=4) as fpool, \
             tc.tile_pool(name="ps_sc", bufs=2, space="PSUM") as ps_sc, \
             tc.tile_pool(name="ps_q", bufs=2, space="PSUM") as ps_q, \
             tc.tile_pool(name="ps_av", bufs=2, space="PSUM") as ps_av:

            # ---- constants / weights in SBUF ----
            tri = cpool.tile([128, 256], BF16, tag="tri", name="tri")
            bvm = cpool.tile([128, HPC * O], BF16, tag="bvm", name="bvm")

            WQ = cpool.tile([128, N_DT, APC], BF16, tag="WQ", name="WQ")
            WK = cpool.tile([128, N_DT, 128], BF16, tag="WK", name="WK")
            WV = cpool.tile([128, N_DT, APC], BF16, tag="WV", name="WV")
            wq_sb = [WQ[:, dt, :] for dt in range(N_DT)]
            wk_sb = [WK[:, dt, :] for dt in range(N_DT)]
            wv_sb = [WV[:, dt, :] for dt in range(N_DT)]
            bq_sb = [cpool.tile([128, 1], F32, tag=f"bq{at}", name=f"bq{at}")
                     for at in range(2)]

            # ---- x^T half tiles (DMA'd in [128,512] chunk pieces) ----
            xth = [[ppool.tile([128, 1024], BF16, tag=f"xt{dt}_{sp}",
                               name=f"xt{dt}_{sp}") for sp in range(2)]
                   for dt in range(N_DT)]

            def xchunk(dt, c):
                """[128,512] view of x^T chunk c for contraction tile dt"""
                sp, so = c // 2, (c % 2) * 512
                return xth[dt][sp][:, so:so + 512]

            def xchunk_dram(dt, c):
                return xT_d[c, dt * 128:(dt + 1) * 128, :]

            # PE warm-up input: memset tile, no DMA dependency, so the
            # warm-up spans the input-DMA wait and keeps the HAM clock
            # gate open (first real matmuls then run at 2.4GHz).
            wu_in = cpool.tile([128, 128], BF16, tag="wu_in", name="wu_in")
            nc.gpsimd.memset(wu_in[:, :], 1.0)

            # dummy exp pulls the ACT exp-table load off the critical path
            tw = fpool.tile([128, 1], F32, tag="tw", name="tw")
            nc.scalar.activation(out=tw[:, :], in_=wu_in[:, 0:1],
                                 func=AF.Exp, scale=1.0)

            # input DMAs: ~128KB granularity (per-DMA issue costs ~650ns
            # of the issuing engine; per-queue BW caps each transfer),
            # head-critical first: x chunk 0, wk2, wq, x chunk 3.
            def dmaj(dram2d, i, n):
                rows = slice(i * 512 // n, (i + 1) * 512 // n)
                v = dram2d[rows].rearrange("(t p) c -> p t c", p=128)
                return v[:, 0, :] if n == 4 else v

            SY, SC, GP = nc.sync, nc.scalar, nc.gpsimd
            order = [(bq_sb[at][:, :], bq_d[at]) for at in range(2)]
            for dt in range(N_DT):
                order.append((xchunk(dt, 0), xchunk_dram(dt, 0)))
            for dt in range(N_DT):
                order.append((wk_sb[dt], dmaj(wk2_d, dt, 4)))
            for dt in range(N_DT):
                order.append((wq_sb[dt], dmaj(wq_d, dt, 4)))
            for dt in range(N_DT):
                order.append((xchunk(dt, 3), xchunk_dram(dt, 3)))
            for dt in range(N_DT):
                order.append((xchunk(dt, 1), xchunk_dram(dt, 1)))
            for dt in range(N_DT):
                order.append((xchunk(dt, 2), xchunk_dram(dt, 2)))
            order += [(tri[:, :], tri_d[:, :]), (bvm[:, :], bvm_d[:, :])]
            for dt in range(N_DT):
                order.append((wv_sb[dt], dmaj(wv_d, dt, 4)))
            for i, (dst, srcap) in enumerate(order):
                [SY, SC, GP][i % 3].dma_start(out=dst, in_=srcap)

            wu = ps_sc.tile([128, 128], F32, tag="sc", name="wu")
            for i in range(40):
                nc.tensor.matmul(out=wu[:, :], lhsT=wu_in[:, :],
                                 rhs=wu_in[:, :], start=True, stop=True)

            # ---- persistent projection outputs ----
            qt = [ppool.tile([128, S], BF16, tag=f"qt{at}", name=f"qt{at}")
                  for at in range(2)]
            kt = ppool.tile([128, S], BF16, tag="kt", name="kt")
            vt = [ppool.tile([128, HPC * VB], BF16, tag=f"v{st}",
                             name=f"v{st}") for st in range(N_ST)]
            for st in range(N_ST):
                v3o = vt[st][:, :].rearrange("p (h c) -> p h c",
                                             h=HPC)[:, :, O:VB]
                nc.gpsimd.memset(v3o, 1.0)

            def qt_proj(at, c):
                qps = ps_q.tile([128, QCH], F32, tag="q", name="qps")
                for dt in range(N_DT):
                    nc.tensor.matmul(
                        out=qps[:, :],
                        lhsT=wq_sb[dt][:, at * 128:(at + 1) * 128],
                        rhs=xchunk(dt, c),
                        start=(dt == 0), stop=(dt == N_DT - 1))
                # bias-add on ACT (Identity w/ per-partition bias AP):
                # keeps the DVE queue shallow so PSUM recycling and av
                # copies are never delayed
                nc.scalar.activation(
                    out=qt[at][:, c * QCH:(c + 1) * QCH], in_=qps[:, :],
                    func=AF.Identity, bias=bq_sb[at][:, 0:1], scale=1.0)

            def kt_proj(c):
                kps = ps_q.tile([128, QCH], F32, tag="q", name="kps")
                for dt in range(N_DT):
                    nc.tensor.matmul(out=kps[:, :], lhsT=wk_sb[dt][:, :],
                                     rhs=xchunk(dt, c),
                                     start=(dt == 0), stop=(dt == N_DT - 1))
                nc.scalar.copy(kt[:, c * QCH:(c + 1) * QCH], kps[:, :])

            def v_proj(st):
                vps = ps_q.tile([128, QCH], F32, tag="q", name="vps")
                for dt in range(N_DT):
                    nc.tensor.matmul(
                        out=vps[:, 0:APC],
                        lhsT=xth[dt][st // 8][:, (st % 8) * 128:
                                              (st % 8) * 128 + 128],
                        rhs=wv_sb[dt][:, :],
                        start=(dt == 0), stop=(dt == N_DT - 1))
                v3 = vt[st][:, :].rearrange("p (h c) -> p h c",
                                            h=HPC)[:, :, 0:O]
                p3 = vps[:, 0:APC].rearrange("p (h c) -> p h c", h=HPC)
                b3 = bvm[:, :].rearrange("p (h c) -> p h c", h=HPC)
                nc.vector.tensor_add(out=v3, in0=p3, in1=b3)

            # ---- attention primitives ----
            def sc_blk(at, c, kj):
                """one score pair-block: 2 row-tiled matmuls + exp(+mask)"""
                diag = kj >= 4 * c
                vs = 128 * (kj - 4 * c) if diag else 0
                sc_t = ps_sc.tile([128, 1024], F32, tag="sc", name="sct")
                for hh in range(2):
                    pl = slice(64 * hh, 64 * hh + 64)
                    nc.tensor.matmul(
                        out=sc_t[:, 512 * hh + vs:512 * hh + 512],
                        lhsT=kt[pl, 128 * kj:128 * (kj + 1)],
                        rhs=qt[at][pl, c * QCH + vs:(c + 1) * QCH],
                        start=True, stop=True)
                atn = apool.tile([128, 1024], BF16, tag="atn", name="atn")
                if vs == 0 and kj in DVE_EXP.get((at, c), ()):
                    # quadratic exp on the DVE: only the PSUM-freeing
                    # cast runs inline; the SBUF-only poly ops are
                    # deferred to the unit end so they never delay
                    # PSUM-recycling work in the in-order DVE queue.
                    sb = apool.tile([128, 1024], BF16, tag="atn",
                                    name="sb")
                    nc.vector.tensor_copy(sb[:, :], sc_t[:, :])
                    poly_pend.append((sb, atn))
                elif vs == 0:
                    nc.scalar.activation(out=atn[:, :], in_=sc_t[:, :],
                                         func=AF.Exp, scale=1.0)
                else:
                    i3 = sc_t[:, :].rearrange("p (h q) -> p h q",
                                              h=2)[:, :, vs:512]
                    o3 = atn[:, :].rearrange("p (h q) -> p h q",
                                             h=2)[:, :, vs:512]
                    nc.scalar.activation(out=o3, in_=i3, func=AF.Exp,
                                         scale=1.0)
                if diag:
                    a3 = atn[:, :].rearrange("p (h q) -> p h q",
                                             h=2)[:, :, vs:vs + 128]
                    t3 = tri[:, :].rearrange("p (h q) -> p h q", h=2)
                    nc.vector.tensor_mul(a3, a3, t3)
                return atn

            def av_pieces(at, c, hh):
                """V-weighted accumulation for one head, split into
                ~4-matmul pieces so filler slots stay fine-grained;
                the last piece copies out of PSUM and DMAs."""
                nkj = 4 * (c + 1)
                h = 2 * at + hh
                av_t = []

                def piece(k0, k1):
                    def go():
                        if k0 == 0:
                            av_t.append(ps_av.tile([128, QCH], F32,
                                                   tag="av", name="av"))
                        av = av_t[0]
                        blks = atns[(at, c)]
                        for kj in range(k0, k1):
                            diag = kj >= 4 * c
                            vs = 128 * (kj - 4 * c) if diag else 0
                            nc.tensor.matmul(
                                out=av[0:VB, vs:512],
                                lhsT=vt[kj][:, h * VB:(h + 1) * VB],
                                rhs=blks[kj][:, 512 * hh + vs:
                                             512 * hh + 512],
                                start=(kj == 0), stop=(kj == nkj - 1))
                        if k1 == nkj:
                            ov = fpool.tile([VB, QCH], F32, tag="ov",
                                            name="ov")
                            nc.vector.tensor_copy(ov[:, :], av[0:VB, :])
                            [SY, GP][hh].dma_start(out=out_d[h, c],
                                                   in_=ov[:, :])
                    return go
                return [piece(k0, min(k0 + 4, nkj))
                        for k0 in range(0, nkj, 4)]

            # ---- interleaved schedule ----
            atns = {}
            poly_pend = []

            def flush_poly():
                """deferred DVE quadratic: u=s/2+1; w=s*u; atn=w+1"""
                for sb, atn in poly_pend:
                    u = apool.tile([128, 1024], BF16, tag="atn", name="u")
                    nc.vector.tensor_scalar(
                        out=u[:, :], in0=sb[:, :], scalar1=0.5,
                        scalar2=1.0, op0=AluOpType.mult, op1=AluOpType.add)
                    w = apool.tile([128, 1024], BF16, tag="atn", name="w")
                    nc.vector.tensor_tensor(out=w[:, :], in0=sb[:, :],
                                            in1=u[:, :], op=AluOpType.mult)
                    nc.vector.tensor_scalar_add(out=atn[:, :], in0=w[:, :],
                                                scalar1=1.0)
                poly_pend.clear()

            def unit(at, c, fillers, per=2):
                """score-blocks of one unit with PE fillers woven in"""
                nkj = 4 * (c + 1)
                blks = atns.setdefault((at, c), [])
                fi = iter(fillers)
                for kj in range(nkj):
                    blks.append(sc_blk(at, c, kj))
                    if kj % per == per - 1:
                        f = next(fi, None)
                        if f is not None:
                            f()
                for f in fi:
                    f()
                flush_poly()

            def AVF(at, c, hh):
                ps = av_pieces(at, c, hh)

                def go():
                    for p in ps:
                        p()
                return go

            def AVH(at, c, hh):
                """chain split in two half fillers (big chains only)"""
                ps = av_pieces(at, c, hh)
                n = len(ps)

                def half(sl):
                    def go():
                        for p in sl:
                            p()
                    return go
                return half(ps[:(n + 1) // 2]), half(ps[(n + 1) // 2:])

            kt_proj(0)
            qt_proj(0, 3)
            unit(0, 3, [lambda: kt_proj(1), lambda: qt_proj(1, 3),
                        lambda: kt_proj(2), lambda: v_proj(0),
                        lambda: kt_proj(3), lambda: qt_proj(0, 2),
                        lambda: v_proj(1), lambda: v_proj(2)])
            unit(1, 3, [lambda: qt_proj(1, 2), lambda: v_proj(3),
                        lambda: v_proj(4), lambda: v_proj(5),
                        lambda: v_proj(6), lambda: v_proj(7),
                        lambda: v_proj(8), lambda: v_proj(9)])
            unit(0, 2, [lambda: v_proj(10), lambda: v_proj(11),
                        lambda: v_proj(12), lambda: v_proj(13),
                        lambda: v_proj(14), lambda: v_proj(15)])
            unit(1, 2, [AVF(0, 3, 0), lambda: qt_proj(0, 1),
                        AVF(0, 3, 1), lambda: qt_proj(1, 1)])
            unit(0, 1, [AVF(1, 3, 0), lambda: qt_proj(0, 0),
                        AVF(1, 3, 1)])
            unit(1, 1, [AVF(0, 2, 0), lambda: qt_proj(1, 0),
                        AVF(0, 2, 1)])
            unit(0, 0, [AVF(1, 2, 0), AVF(1, 2, 1), AVF(0, 1, 0),
                        AVF(0, 1, 1)], per=1)
            unit(1, 0, [AVF(1, 1, 0), AVF(1, 1, 1), AVF(0, 0, 0),
                        AVF(0, 0, 1)], per=1)
            for f in [AVF(1, 0, 0), AVF(1, 0, 1)]:
                f()

    nc.compile()
    return nc


_NC = None
LAST_RESULTS = None


def make_in_maps(x, Wq, bq, Wk, Wv, bv):
    in_maps = []
    wk2 = np.concatenate([Wk * RS, Wk * RS], axis=1)  # [D, 128]
    for c in range(NCORES):
        b, g = c % 4, c // 4
        cols = slice(g * APC, (g + 1) * APC)
        bvm = np.ascontiguousarray(np.broadcast_to(
            np.asarray(bv[cols], dtype=np.float32).reshape(1, APC),
            (128, APC))).astype(BF_NP)
        xTb = x[b].T  # [D, S]
        xT4 = np.stack([xTb[:, c4 * 512:(c4 + 1) * 512]
                        for c4 in range(NCH)])  # [NCH, D, 512]
        in_maps.append({
            "xT4": np.ascontiguousarray(xT4).astype(BF_NP),
            "wq": np.ascontiguousarray(Wq[:, cols] * RS).astype(BF_NP),
            "bq": np.ascontiguousarray(
                (bq[cols] * RS).astype(np.float32).reshape(2, 128, 1)),
            "wk2": np.ascontiguousarray(wk2).astype(BF_NP),
            "wv": np.ascontiguousarray(Wv[:, cols]).astype(BF_NP),
            "bvm": bvm,
        })
    return in_maps


def gather_out(results):
    out = np.empty((B, S, H * O), dtype=np.float32)
    for c in range(NCORES):
        b, g = c % 4, c // 4
        oc = results[c]["out"]          # [HPC, NCH, VB, QCH] fp32
        for h in range(HPC):
            col = g * APC + h * O
            for ch in range(NCH):
                blk = oc[h, ch]         # [65, QCH]
                out[b, ch * QCH:(ch + 1) * QCH, col:col + O] = \
                    (blk[0:O, :] / blk[O:O + 1, :]).T
    return out


def kernel(**inputs):
    global _NC, LAST_RESULTS
    x = np.asarray(inputs["x"], dtype=np.float32)
    Wq = np.asarray(inputs["Wq"], dtype=np.float32)
    bq = np.asarray(inputs["bq"], dtype=np.float32)
    Wk = np.asarray(inputs["Wk"], dtype=np.float32)
    Wv = np.asarray(inputs["Wv"], dtype=np.float32)
    bv = np.asarray(inputs["bv"], dtype=np.float32)

    if _NC is None:
        _NC = build()

    in_maps = make_in_maps(x, Wq, bq, Wk, Wv, bv)
    res = run_bass_kernel_spmd(_NC, in_maps, core_ids=list(range(NCORES)))
    LAST_RESULTS = res
    return gather_out(res.results)


# revision 33
# speedup vs baseline: 1.0132x; 1.0132x over previous
"""Multi-head attention (shared key head) on 8 TRN2 NeuronCores — v4.

Sharding: core c handles batch b = c % 4 and head group g = c // 4
(heads 4g..4g+3).  Per-core weights are sliced on host; x is
pre-transposed (and bf16-cast) on host.  sqrt(scale) is folded into
Wq/bq/Wk on host so scores arrive pre-scaled.

v4 (final): the PE matmul stream (~89us busy: AV is 65/128-col
utilisation — irreducible, each attn element feeds only 65 outputs
per head) and the scalar-engine exp stream (ACT, 1 col/cycle @
1.2GHz, ~70us unassisted) are co-critical, so the kernel is one
globally interleaved software pipeline:
  - units ordered largest-exp-first: (0,3),(1,3),(0,2),(1,2),...,
    so ACT runs flat-out from ~t=9us; projections and AV chains are
    woven between score-blocks as PE filler at the pace ACT drains
    score PSUM tiles (ps_sc double-buffered; the in-order PE queue
    self-regulates).  AV chain for unit X is emitted >=2 units later
    (its exps are certainly done -> no PE stall); tail AV chains are
    pulled into the last units' filler slots.
  - six early off-diag blocks run quadratic exp on the DVE
    (exp(s) ~= (1+s) + s^2/2, |s|<=0.29 so err < 4e-3): the
    PSUM-freeing cast runs inline, the SBUF-only poly ops are
    deferred to the unit end so the in-order DVE queue never delays
    PSUM recycling; remaining PSUM->SBUF bias/copy work rides ACT
    (Identity with per-partition bias AP / Copy) to keep DVE shallow.
  - softmax normalisation on the host: the ones-column of V gives the
    denominator as row 64 of the AV accumulator; device ships raw
    [65, q] fp32 per (head, chunk); host divides + transposes.
    (fp8 anywhere is fatal here: dot-product relative error does not
    average down with K, and the rel-err gate is max-norm.)
  - x is DMA'd chunk-major (one contiguous 128KB read per (chunk,
    dt)), bq first, head-critical chunks 0/3 before 1/2; PE warm-up
    reads a memset tile so it needs no DMA and spans the input wait,
    keeping the HAM clock gate at 2.4GHz.
"""

import math
import numpy as np
import ml_dtypes

import concourse.bass as bass
import concourse.mybir as mybir
import concourse.tile as tile
from concourse import bacc
from concourse.bass_utils import run_bass_kernel_spmd
from concourse.alu_op_type import AluOpType

B, S, D = 4, 2048, 512
H, A, O = 8, 64, 64
NCORES = 8
HPC = 4                # heads per core
APC = HPC * A          # 256 projection cols per core
VB = O + 1             # V block: 64 out + 1 ones column (denominator)
SCALE = 1.0 / math.sqrt(S)
RS = math.sqrt(SCALE)  # folded into wq, bq, wk on host

F32 = mybir.dt.float32
BF16 = mybir.dt.bfloat16
AF = mybir.ActivationFunctionType
BF_NP = ml_dtypes.bfloat16

QCH = 512              # q-chunk width
NCH = S // QCH         # 4
N_DT = D // 128        # 4 contraction tiles
N_ST = S // 128        # 16 s-tiles / k-tiles

# off-diagonal score blocks whose exp runs on the DVE (quadratic)
DVE_EXP = {(0, 3): (0, 4, 8), (1, 3): (0, 4, 8)}


def build():
    nc = bacc.Bacc("TRN2", target_bir_lowering=False, debug=False,
                   num_devices=NCORES)

    xT_d = nc.dram_tensor("xT4", [NCH, D, 512], BF16,
                          kind="ExternalInput").ap()
    wq_d = nc.dram_tensor("wq", [D, APC], BF16, kind="ExternalInput").ap()
    bq_d = nc.dram_tensor("bq", [2, 128, 1], F32, kind="ExternalInput").ap()
    wk2_d = nc.dram_tensor("wk2", [D, 128], BF16, kind="ExternalInput").ap()
    wv_d = nc.dram_tensor("wv", [D, APC], BF16, kind="ExternalInput").ap()
    bvm_d = nc.dram_tensor("bvm", [128, HPC * O], BF16,
                           kind="ExternalInput").ap()
    out_d = nc.dram_tensor("out", [HPC, NCH, VB, QCH], F32,
                           kind="ExternalOutput").ap()

    # causal boundary-block mask: tri[k, q] = 1 where q >= k (valid),
    # duplicated for both heads of a pair ([128, 2*128] -> [p, 2, 128])
    tri_np = (np.arange(128)[None, :] >= np.arange(128)[:, None])
    tri2_np = np.concatenate([tri_np, tri_np], axis=1)
    tri_d = nc.inline_tensor(tri2_np.astype(BF_NP), "tri").ap()

    with tile.TileContext(nc) as tc:
        with tc.tile_pool(name="const", bufs=1) as cpool, \
             tc.tile_pool(name="persist", bufs=1) as ppool, \
             tc.tile_pool(name="attn", bufs=48) as apool, \
             tc.tile_pool(name="fin", bufs=4) as fpool, \
             tc.tile_pool(name="ps_sc", bufs=2, space="PSUM") as ps_sc, \
             tc.tile_pool(name="ps_q", bufs=2, space="PSUM") as ps_q, \
             tc.tile_pool(name="ps_av", bufs=2, space="PSUM") as ps_av:

            # ---- constants / weights in SBUF ----
            tri = cpool.tile([128, 256], BF16, tag="tri", name="tri")
            bvm = cpool.tile([128, HPC * O], BF16, tag="bvm", name="bvm")

            WQ = cpool.tile([128, N_DT, APC], BF16, tag="WQ", name="WQ")
            WK = cpool.tile([128, N_DT, 128], BF16, tag="WK", name="WK")
            WV = cpool.tile([128, N_DT, APC], BF16, tag="WV", name="WV")
            wq_sb = [WQ[:, dt, :] for dt in range(N_DT)]
            wk_sb = [WK[:, dt, :] for dt in range(N_DT)]
            wv_sb = [WV[:, dt, :] for dt in range(N_DT)]
            bq_sb = [cpool.tile([128, 1], F32, tag=f"bq{at}", name=f"bq{at}")
                     for at in range(2)]

            # ---- x^T half tiles (DMA'd in [128,512] chunk pieces) ----
            xth = [[ppool.tile([128, 1024], BF16, tag=f"xt{dt}_{sp}",
                               name=f"xt{dt}_{sp}") for sp in range(2)]
                   for dt in range(N_DT)]

            def xchunk(dt, c):
                """[128,512] view of x^T chunk c for contraction tile dt"""
                sp, so = c // 2, (c % 2) * 512
                return xth[dt][sp][:, so:so + 512]

            def xchunk_dram(dt, c):
                return xT_d[c, dt * 128:(dt + 1) * 128, :]

            # PE warm-up input: memset tile, no DMA dependency, so the
            # warm-up spans the input-DMA wait and keeps the HAM clock
            # gate open (first real matmuls then run at 2.4GHz).
            wu_in = cpool.tile([128, 128], BF16, tag="wu_in", name="wu_in")
            nc.gpsimd.memset(wu_in[:, :], 1.0)

            # dummy exp pulls the ACT exp-table load off the critical path
            tw = fpool.tile([128, 1], F32, tag="tw", name="tw")
            nc.scalar.activation(out=tw[:, :], in_=wu_in[:, 0:1],
                                 func=AF.Exp, scale=1.0)

            # input DMAs: ~128KB granularity (per-DMA issue costs ~650ns
            # of the issuing engine; per-queue BW caps each transfer),
            # head-critical first: x chunk 0, wk2, wq, x chunk 3.
            def dmaj(dram2d, i, n):
                rows = slice(i * 512 // n, (i + 1) * 512 // n)
                v = dram2d[rows].rearrange("(t p) c -> p t c", p=128)
                return v[:, 0, :] if n == 4 else v

            SY, SC, GP = nc.sync, nc.scalar, nc.gpsimd
            order = [(bq_sb[at][:, :], bq_d[at]) for at in range(2)]
            for dt in range(N_DT):
                order.append((xchunk(dt, 0), xchunk_dram(dt, 0)))
            for dt in range(N_DT):
                order.append((wk_sb[dt], dmaj(wk2_d, dt, 4)))
            for dt in range(N_DT):
                order.append((wq_sb[dt], dmaj(wq_d, dt, 4)))
            for dt in range(N_DT):
                order.append((xchunk(dt, 3), xchunk_dram(dt, 3)))
            for dt in range(N_DT):
                order.append((xchunk(dt, 1), xchunk_dram(dt, 1)))
            for dt in range(N_DT):
                order.append((xchunk(dt, 2), xchunk_dram(dt, 2)))
            order += [(tri[:, :], tri_d[:, :]), (bvm[:, :], bvm_d[:, :])]
            for dt in range(N_DT):
                order.append((wv_sb[dt], dmaj(wv_d, dt, 4)))
            for i, (dst, srcap) in enumerate(order):
                [SY, SC, GP][i % 3].dma_start(out=dst, in_=srcap)

            wu = ps_sc.tile([128, 128], F32, tag="sc", name="wu")
            for i in range(40):
                nc.tensor.matmul(out=wu[:, :], lhsT=wu_in[:, :],
                                 rhs=wu_in[:, :], start=True, stop=True)

            # ---- persistent projection outputs ----
            qt = [ppool.tile([128, S], BF16, tag=f"qt{at}", name=f"qt{at}")
                  for at in range(2)]
            kt = ppool.tile([128, S], BF16, tag="kt", name="kt")
            vt = [ppool.tile([128, HPC * VB], BF16, tag=f"v{st}",
                             name=f"v{st}") for st in range(N_ST)]
            for st in range(N_ST):
                v3o = vt[st][:, :].rearrange("p (h c) -> p h c",
                                             h=HPC)[:, :, O:VB]
                nc.gpsimd.memset(v3o, 1.0)

            def qt_proj(at, c):
                qps = ps_q.tile([128, QCH], F32, tag="q", name="qps")
                for dt in range(N_DT):
                    nc.tensor.matmul(
                        out=qps[:, :],
                        lhsT=wq_sb[dt][:, at * 128:(at + 1) * 128],
                        rhs=xchunk(dt, c),
                        start=(dt == 0), stop=(dt == N_DT - 1))
                # bias-add on ACT (Identity w/ per-partition bias AP):
                # keeps the DVE queue shallow so PSUM recycling and av
                # copies are never delayed
                nc.scalar.activation(
                    out=qt[at][:, c * QCH:(c + 1) * QCH], in_=qps[:, :],
                    func=AF.Identity, bias=bq_sb[at][:, 0:1], scale=1.0)

            def kt_proj(c):
                kps = ps_q.tile([128, QCH], F32, tag="q", name="kps")
                for dt in range(N_DT):
                    nc.tensor.matmul(out=kps[:, :], lhsT=wk_sb[dt][:, :],
                                     rhs=xchunk(dt, c),
                                     start=(dt == 0), stop=(dt == N_DT - 1))
                nc.scalar.copy(kt[:, c * QCH:(c + 1) * QCH], kps[:, :])

            def v_proj(st):
                vps = ps_q.tile([128, QCH], F32, tag="q", name="vps")
                for dt in range(N_DT):
                    nc.tensor.matmul(
                        out=vps[:, 0:APC],
                        lhsT=xth[dt][st // 8][:, (st % 8) * 128:
                                              (st % 8) * 128 + 128],
                        rhs=wv_sb[dt][:, :],
                        start=(dt == 0), stop=(dt == N_DT - 1))
                v3 = vt[st][:, :].rearrange("p (h c) -> p h c",
                                            h=HPC)[:, :, 0:O]
                p3 = vps[:, 0:APC].rearrange("p (h c) -> p h c", h=HPC)
                b3 = bvm[:, :].rearrange("p (h c) -> p h c", h=HPC)
                nc.vector.tensor_add(out=v3, in0=p3, in1=b3)

            # ---- attention primitives ----
            def sc_blk(at, c, kj):
                """one score pair-block: 2 row-tiled matmuls + exp(+mask)"""
                diag = kj >= 4 * c
                vs = 128 * (kj - 4 * c) if diag else 0
                sc_t = ps_sc.tile([128, 1024], F32, tag="sc", name="sct")
                for hh in range(2):
                    pl = slice(64 * hh, 64 * hh + 64)
                    nc.tensor.matmul(
                        out=sc_t[:, 512 * hh + vs:512 * hh + 512],
                        lhsT=kt[pl, 128 * kj:128 * (kj + 1)],
                        rhs=qt[at][pl, c * QCH + vs:(c + 1) * QCH],
                        start=True, stop=True)
                atn = apool.tile([128, 1024], BF16, tag="atn", name="atn")
                if vs == 0 and kj in DVE_EXP.get((at, c), ()):
                    # quadratic exp on the DVE: only the PSUM-freeing
                    # cast runs inline; the SBUF-only poly ops are
                    # deferred to the unit end so they never delay
                    # PSUM-recycling work in the in-order DVE queue.
                    sb = apool.tile([128, 1024], BF16, tag="atn",
                                    name="sb")
                    nc.vector.tensor_copy(sb[:, :], sc_t[:, :])
                    poly_pend.append((sb, atn))
                elif vs == 0:
                    nc.scalar.activation(out=atn[:, :], in_=sc_t[:, :],
                                         func=AF.Exp, scale=1.0)
                else:
                    i3 = sc_t[:, :].rearrange("p (h q) -> p h q",
                                              h=2)[:, :, vs:512]
                    o3 = atn[:, :].rearrange("p (h q) -> p h q",
                                             h=2)[:, :, vs:512]
                    nc.scalar.activation(out=o3, in_=i3, func=AF.Exp,
                                         scale=1.0)
                if diag:
                    a3 = atn[:, :].rearrange("p (h q) -> p h q",
                                             h=2)[:, :, vs:vs + 128]
                    t3 = tri[:, :].rearrange("p (h q) -> p h q", h=2)
                    nc.vector.tensor_mul(a3, a3, t3)
                return atn

            def av_pieces(at, c, hh):
                """V-weighted accumulation for one head, split into
                ~4-matmul pieces so filler slots stay fine-grained;
                the last piece copies out of PSUM and DMAs."""
                nkj = 4 * (c + 1)
                h = 2 * at + hh
                av_t = []

                def piece(k0, k1):
                    def go():
                        if k0 == 0:
                            av_t.append(ps_av.tile([128, QCH], F32,
                                                   tag="av", name="av"))
                        av = av_t[0]
                        blks = atns[(at, c)]
                        for kj in range(k0, k1):
                            diag = kj >= 4 * c
                            vs = 128 * (kj - 4 * c) if diag else 0
                            nc.tensor.matmul(
                                out=av[0:VB, vs:512],
                                lhsT=vt[kj][:, h * VB:(h + 1) * VB],
                                rhs=blks[kj][:, 512 * hh + vs:
                                             512 * hh + 512],
                                start=(kj == 0), stop=(kj == nkj - 1))
                        if k1 == nkj:
                            ov = fpool.tile([VB, QCH], F32, tag="ov",
                                            name="ov")
                            nc.vector.tensor_copy(ov[:, :], av[0:VB, :])
                            [SY, GP][hh].dma_start(out=out_d[h, c],
                                                   in_=ov[:, :])
                    return go
                return [piece(k0, min(k0 + 4, nkj))
                        for k0 in range(0, nkj, 4)]

            # ---- interleaved schedule ----
            atns = {}
            poly_pend = []

            def flush_poly():
                """deferred DVE quadratic: u=s/2+1; w=s*u; atn=w+1"""
                for sb, atn in poly_pend:
                    u = apool.tile([128, 1024], BF16, tag="atn", name="u")
                    nc.vector.tensor_scalar(
                        out=u[:, :], in0=sb[:, :], scalar1=0.5,
                        scalar2=1.0, op0=AluOpType.mult, op1=AluOpType.add)
                    w = apool.tile([128, 1024], BF16, tag="atn", name="w")
                    nc.vector.tensor_tensor(out=w[:, :], in0=sb[:, :],
                                            in1=u[:, :], op=AluOpType.mult)
                    nc.vector.tensor_scalar_add(out=atn[:, :], in0=w[:, :],
                                                scalar1=1.0)
                poly_pend.clear()

            def unit(at, c, fillers, per=2):
                """score-blocks of one unit with PE fillers woven in"""
                nkj = 4 * (c + 1)
                blks = atns.setdefault((at, c), [])
                fi = iter(fillers)
                for kj in range(nkj):
                    blks.append(sc_blk(at, c, kj))
                    if kj % per == per - 1:
                        f = next(fi, None)
                        if f is not None:
                            f()
                for f in fi:
                    f()
                flush_poly()

            def AVF(at, c, hh):
                ps = av_pieces(at, c, hh)

                def go():
                    for p in ps:
                        p()
                return go

            def AVH(at, c, hh):
                """chain split in two half fillers (big chains only)"""
                ps = av_pieces(at, c, hh)
                n = len(ps)

                def half(sl):
                    def go():
                        for p in sl:
                            p()
                    return go
                return half(ps[:(n + 1) // 2]), half(ps[(n + 1) // 2:])

            kt_proj(0)
            qt_proj(0, 3)
            unit(0, 3, [lambda: kt_proj(1), lambda: qt_proj(1, 3),
                        lambda: kt_proj(2), lambda: v_proj(0),
                        lambda: kt_proj(3), lambda: qt_proj(0, 2),
                        lambda: v_proj(1), lambda: v_proj(2)])
            unit(1, 3, [lambda: qt_proj(1, 2), lambda: v_proj(3),
                        lambda: v_proj(4), lambda: v_proj(5),
                        lambda: v_proj(6), lambda: v_proj(7),
                        lambda: v_proj(8), lambda: v_proj(9)])
            unit(0, 2, [lambda: v_proj(10), lambda: v_proj(11),
                        lambda: v_proj(12), lambda: v_proj(13),
                        lambda: v_proj(14), lambda: v_proj(15)])
            unit(1, 2, [AVF(0, 3, 0), lambda: qt_proj(0, 1),
                        AVF(0, 3, 1), lambda: qt_proj(1, 1)])
            unit(0, 1, [AVF(1, 3, 0), lambda: qt_proj(0, 0),
                        AVF(1, 3, 1)])
            unit(1, 1, [AVF(0, 2, 0), lambda: qt_proj(1, 0),
                        AVF(0, 2, 1)])
            unit(0, 0, [AVF(1, 2, 0), AVF(1, 2, 1), AVF(0, 1, 0),
                        AVF(0, 1, 1)], per=1)
            unit(1, 0, [AVF(1, 1, 0), AVF(1, 1, 1), AVF(0, 0, 0),
                        AVF(0, 0, 1)], per=1)
            for f in [AVF(1, 0, 0), AVF(1, 0, 1)]:
                f()

    nc.compile()
    return nc


_NC = None
LAST_RESULTS = None


def make_in_maps(x, Wq, bq, Wk, Wv, bv):
    in_maps = []
    wk2 = np.concatenate([Wk * RS, Wk * RS], axis=1)  # [D, 128]
    for c in range(NCORES):
        b, g = c % 4, c // 4
        cols = slice(g * APC, (g + 1) * APC)
        bvm = np.ascontiguousarray(np.broadcast_to(
            np.asarray(bv[cols], dtype=np.float32).reshape(1, APC),
            (128, APC))).astype(BF_NP)
        xTb = x[b].T  # [D, S]
        xT4 = np.stack([xTb[:, c4 * 512:(c4 + 1) * 512]
                        for c4 in range(NCH)])  # [NCH, D, 512]
        in_maps.append({
            "xT4": np.ascontiguousarray(xT4).astype(BF_NP),
            "wq": np.ascontiguousarray(Wq[:, cols] * RS).astype(BF_NP),
            "bq": np.ascontiguousarray(
                (bq[cols] * RS).astype(np.float32).reshape(2, 128, 1)),
            "wk2": np.ascontiguousarray(wk2).astype(BF_NP),
            "wv": np.ascontiguousarray(Wv[:, cols]).astype(BF_NP),
            "bvm": bvm,
        })
    return in_maps


def gather_out(results):
    out = np.empty((B, S, H * O), dtype=np.float32)
    for c in range(NCORES):
        b, g = c % 4, c // 4
        oc = results[c]["out"]          # [HPC, NCH, VB, QCH] fp32
        for h in range(HPC):
            col = g * APC + h * O
            for ch in range(NCH):
                blk = oc[h, ch]         # [65, QCH]
                out[b, ch * QCH:(ch + 1) * QCH, col:col + O] = \
                    (blk[0:O, :] / blk[O:O + 1, :]).T
    return out


def kernel(**inputs):
    global _NC, LAST_RESULTS
    x = np.asarray(inputs["x"], dtype=np.float32)
    Wq = np.asarray(inputs["Wq"], dtype=np.float32)
    bq = np.asarray(inputs["bq"], dtype=np.float32)
    Wk = np.asarray(inputs["Wk"], dtype=np.float32)
    Wv = np.asarray(inputs["Wv"], dtype=np.float32)
    bv = np.asarray(inputs["bv"], dtype=np.float32)

    if _NC is None:
        _NC = build()

    in_maps = make_in_maps(x, Wq, bq, Wk, Wv, bv)
    res = run_bass_kernel_spmd(_NC, in_maps, core_ids=list(range(NCORES)))
    LAST_RESULTS = res
    return gather_out(res.results)
